# revision 3
# baseline (speedup 1.0000x reference)
"""Causal self-attention (B=4, T=2048, C=1024, H=16) on 8 Trainium2 NeuronCores.

Sharding: core = (batch b = core//2, head-group g = core%2, 8 heads each).
Per core:
  - QKV projection for its 512 q/k/v channels: fp32r matmuls (full PE rate).
  - Attention per head pair: S^T = K^T.T @ Q^T row-tiled (2 heads share the
    PE array via disjoint 64-row groups), exp on ScalarE (no max subtraction:
    scores are O(1) so exp is safe), causal tril mask multiplied on DVE for
    diagonal tiles only (above-diagonal tiles skipped entirely), PV matmul
    accumulates O'^T = [V | pad]^T-style: V' has the padding mask multiplied
    in and a pad column appended, so row 64 of the PSUM accumulator is the
    softmax denominator (padding handled with zero per-tile cost).
  - Normalize: reciprocal of the denominator row, gpsimd partition-broadcast,
    DVE multiply into y^T.
  - Output projection rows slice -> partial [T, C] output.
Host: transposes x per batch, slices Wqkv/Wproj by head group, sums the two
partials per batch and adds bproj.
"""

import os
import sys

for _p in ("/opt/trn_rl_repo",):
    if _p not in sys.path:
        sys.path.append(_p)

import numpy as np

B, T, C = 4, 2048, 1024
H, D = 16, 64
HPC = 8          # heads per core
GC = HPC * D     # 512 channels per core
N_CORES = 8
P = 128
NT = T // 512    # 4  q-tiles / n-slices of 512
MT = GC // 128   # 4  m-tiles (head pairs)
CT = C // 128    # 8  contraction tiles
TT = T // 128    # 16 t-tiles of 128

_cached = {}


def _build():
    import concourse.tile as tile
    from concourse import bacc, mybir

    f32 = mybir.dt.float32
    f32r = mybir.dt.float32r
    AF = mybir.ActivationFunctionType
    MUL = mybir.AluOpType.mult

    nc = bacc.Bacc("TRN2", target_bir_lowering=False, debug=False)

    xT_d = nc.dram_tensor("xT", [C, T], f32, kind="ExternalInput")
    wq_d = nc.dram_tensor("wq", [C, GC], f32, kind="ExternalInput")
    wk_d = nc.dram_tensor("wk", [C, GC], f32, kind="ExternalInput")
    wv_d = nc.dram_tensor("wv", [C, GC], f32, kind="ExternalInput")
    bq_d = nc.dram_tensor("bq", [GC], f32, kind="ExternalInput")
    bk_d = nc.dram_tensor("bk", [GC], f32, kind="ExternalInput")
    bv_d = nc.dram_tensor("bv", [GC], f32, kind="ExternalInput")
    wp_d = nc.dram_tensor("wp", [GC, C], f32, kind="ExternalInput")
    pad_d = nc.dram_tensor("pad", [T], f32, kind="ExternalInput")
    mask_d = nc.dram_tensor("mask", [4, P, 512], f32, kind="ExternalInput")
    out_d = nc.dram_tensor("out", [T, C], f32, kind="ExternalOutput")

    import concourse.bass as bass

    with tile.TileContext(nc) as tc:
        with tc.tile_pool(name="persist", bufs=1) as persist:
            QT = persist.tile([P, MT, T], f32r, tag="QT")
            KT = persist.tile([P, MT, T], f32r, tag="KT")
            Vp = persist.tile([P, TT, HPC, D + 1], f32r, tag="Vp")
            pad_s = persist.tile([P, TT], f32, tag="pad")
            bq_s = persist.tile([P, MT], f32, tag="bq")
            bk_s = persist.tile([P, MT], f32, tag="bk")
            bv_s = persist.tile([P, GC], f32, tag="bv")

            nc.sync.dma_start(pad_s[:], pad_d.rearrange("(tt p) -> p tt", p=P))
            nc.sync.dma_start(bq_s[:], bq_d.rearrange("(m p) -> p m", p=P))
            nc.sync.dma_start(bk_s[:], bk_d.rearrange("(m p) -> p m", p=P))
            # bv broadcast across partitions
            bv_ap = bass.AP(tensor=bv_d[:].tensor, offset=0, ap=[[0, P], [1, GC]])
            nc.sync.dma_start(bv_s[:], bv_ap)
            # Vp pad column: Vp[:, tt, h, 64] = pad[tt*128 + p] for all h
            MUL_ = mybir.AluOpType.mult
            for tt in range(TT):
                nc.vector.memset(Vp[:, tt, :, D:D + 1].bitcast(f32), 1.0)
                nc.vector.tensor_scalar(
                    out=Vp[:, tt, :, D:D + 1], in0=Vp[:, tt, :, D:D + 1],
                    scalar1=pad_s[:, tt:tt + 1], scalar2=None, op0=MUL_)

            xTr = xT_d.rearrange("(c p) t -> p c t", p=P).bitcast(f32r)

            # ---------------- phase 1: V + Q^T + K^T projections ----------
            with tc.tile_pool(name="wpool", bufs=1) as wpool, \
                 tc.tile_pool(name="xpool", bufs=2) as xpool, \
                 tc.tile_pool(name="tpool", bufs=2) as tpool, \
                 tc.tile_pool(name="vqkps", bufs=2, space="PSUM") as vqkps:
                wq_s = wpool.tile([P, CT, GC], f32r, tag="wq")
                wk_s = wpool.tile([P, CT, GC], f32r, tag="wk")
                wv_s = wpool.tile([P, CT, GC], f32r, tag="wv")
                nc.sync.dma_start(wq_s[:], wq_d.rearrange("(c p) n -> p c n", p=P).bitcast(f32r))
                nc.sync.dma_start(wk_s[:], wk_d.rearrange("(c p) n -> p c n", p=P).bitcast(f32r))
                nc.sync.dma_start(wv_s[:], wv_d.rearrange("(c p) n -> p c n", p=P).bitcast(f32r))

                for nt in range(NT):
                    xt_n = xpool.tile([P, CT, 512], f32r, tag="xtn")
                    nc.sync.dma_start(xt_n[:], xTr[:, :, nt * 512:(nt + 1) * 512])
                    # V for 4 t-subtiles of this n-slice
                    for ts in range(4):
                        tt = nt * 4 + ts
                        ps = vqkps.tile([P, GC], f32, tag="ps")
                        for c in range(CT):
                            nc.tensor.matmul(
                                ps[:], xt_n[:, c, ts * P:(ts + 1) * P], wv_s[:, c, :],
                                start=(c == 0), stop=(c == CT - 1))
                        tmp = tpool.tile([P, GC], f32, tag="vtmp")
                        nc.vector.tensor_add(tmp[:], ps[:], bv_s[:])
                        nc.vector.tensor_scalar(
                            out=Vp[:, tt, :, 0:D],
                            in0=tmp[:].rearrange("p (h d) -> p h d", h=HPC),
                            scalar1=pad_s[:, tt:tt + 1], scalar2=None, op0=MUL)
                    # Q^T and K^T m-tiles for this n-slice
                    for W, bias, OUT in ((wq_s, bq_s, QT), (wk_s, bk_s, KT)):
                        for m in range(MT):
                            ps = vqkps.tile([P, 512], f32, tag="ps")
                            for c in range(CT):
                                nc.tensor.matmul(
                                    ps[:], W[:, c, m * P:(m + 1) * P], xt_n[:, c, :],
                                    start=(c == 0), stop=(c == CT - 1))
                            nc.scalar.activation(
                                OUT[:, m, nt * 512:(nt + 1) * 512], ps[:],
                                AF.Identity, bias=bias[:, m:m + 1])

            # ---------------- phase 2: attention + projection -------------
            with tc.tile_pool(name="apool", bufs=1) as apool, \
                 tc.tile_pool(name="ypool", bufs=2) as ypool, \
                 tc.tile_pool(name="ppool", bufs=3) as ppool, \
                 tc.tile_pool(name="bpool", bufs=2) as bpool, \
                 tc.tile_pool(name="prpool", bufs=3) as prpool, \
                 tc.tile_pool(name="attnps", bufs=2, space="PSUM") as attnps:
                masks_s = apool.tile([P, 4, 512], f32r, tag="masks")
                nc.sync.dma_start(masks_s[:], mask_d.rearrange("o p q -> p o q").bitcast(f32r))
                wp_s = apool.tile([P, MT, C], f32r, tag="wp")
                nc.sync.dma_start(wp_s[:], wp_d.rearrange("(m p) n -> p m n", p=P).bitcast(f32r))

                for qt in range(NT):
                    yTq = ypool.tile([P, MT, 512], f32r, tag="yTq")
                    nk = 4 * (qt + 1)
                    for j in range(MT):
                        Oe = attnps.tile([D + 1, 512], f32, tag="Oe")
                        Oo = attnps.tile([D + 1, 512], f32, tag="Oo")
                        for kt in range(nk):
                            Se = attnps.tile([P, 512], f32, tag="S")
                            So = attnps.tile([P, 512], f32, tag="S")
                            nc.tensor.matmul(
                                Se[:], KT[0:D, j, kt * P:(kt + 1) * P],
                                QT[0:D, j, qt * 512:(qt + 1) * 512],
                                start=True, stop=True)
                            nc.tensor.matmul(
                                So[:], KT[D:P, j, kt * P:(kt + 1) * P],
                                QT[D:P, j, qt * 512:(qt + 1) * 512],
                                start=True, stop=True)
                            Pe = ppool.tile([P, 512], f32r, tag="Pe")
                            Po = ppool.tile([P, 512], f32r, tag="Po")
                            nc.scalar.activation(Pe[:], Se[:], AF.Exp, scale=0.125)
                            nc.scalar.activation(Po[:], So[:], AF.Exp, scale=0.125)
                            off = kt - 4 * qt
                            if off >= 0:
                                nc.vector.tensor_mul(Pe[:], Pe[:], masks_s[:, off, :])
                                nc.vector.tensor_mul(Po[:], Po[:], masks_s[:, off, :])
                            nc.tensor.matmul(
                                Oe[:], Vp[:, kt, 2 * j, :], Pe[:],
                                start=(kt == 0), stop=(kt == nk - 1))
                            nc.tensor.matmul(
                                Oo[:], Vp[:, kt, 2 * j + 1, :], Po[:],
                                start=(kt == 0), stop=(kt == nk - 1))
                        lre = bpool.tile([1, 512], f32, tag="lre")
                        lro = bpool.tile([1, 512], f32, tag="lro")
                        nc.vector.reciprocal(lre[0:1, :], Oe[D:D + 1, :])
                        nc.vector.reciprocal(lro[0:1, :], Oo[D:D + 1, :])
                        bce = bpool.tile([P, 512], f32, tag="bce")
                        bco = bpool.tile([P, 512], f32, tag="bco")
                        nc.gpsimd.partition_broadcast(bce[:], lre[0:1, :], channels=P)
                        nc.gpsimd.partition_broadcast(bco[:], lro[0:1, :], channels=P)
                        nc.vector.tensor_mul(yTq[0:D, j, :], Oe[0:D, :], bce[0:D, :])
                        nc.vector.tensor_mul(yTq[D:P, j, :], Oo[0:D, :], bco[D:P, :])
                    # projection for this q block of 512 rows
                    for ts in range(4):
                        tt = qt * 4 + ts
                        for nh in range(2):
                            ps = attnps.tile([P, 512], f32, tag="prps")
                            for cj in range(MT):
                                nc.tensor.matmul(
                                    ps[:], yTq[:, cj, ts * P:(ts + 1) * P],
                                    wp_s[:, cj, nh * 512:(nh + 1) * 512],
                                    start=(cj == 0), stop=(cj == MT - 1))
                            ot = prpool.tile([P, 512], f32, tag="ot")
                            nc.vector.tensor_copy(ot[:], ps[:])
                            nc.sync.dma_start(
                                out_d[tt * P:(tt + 1) * P, nh * 512:(nh + 1) * 512], ot[:])

    nc.compile()
    return nc


def _get_nc():
    if "nc" not in _cached:
        _cached["nc"] = _build()
    return _cached["nc"]


def _make_masks():
    kk = np.arange(P)[:, None]
    qq = np.arange(512)[None, :]
    return np.stack(
        [(kk + off * P <= qq).astype(np.float32) for off in range(4)], axis=0)


def kernel(x, padding_mask, Wqkv, bqkv, Wproj, bproj):
    from concourse.bass_utils import run_bass_kernel_spmd

    x = np.asarray(x, dtype=np.float32)
    padding_mask = np.asarray(padding_mask)
    Wqkv = np.asarray(Wqkv, dtype=np.float32)
    bqkv = np.asarray(bqkv, dtype=np.float32)
    Wproj = np.asarray(Wproj, dtype=np.float32)
    bproj = np.asarray(bproj, dtype=np.float32)
    assert x.shape == (B, T, C), x.shape

    nc = _get_nc()
    masks = _make_masks()
    in_maps = []
    for core in range(N_CORES):
        b, g = divmod(core, 2)
        sl = slice(g * GC, (g + 1) * GC)
        in_maps.append({
            "xT": np.ascontiguousarray(x[b].T),
            "wq": np.ascontiguousarray(Wqkv[:, 0 * C:1 * C][:, sl]),
            "wk": np.ascontiguousarray(Wqkv[:, 1 * C:2 * C][:, sl]),
            "wv": np.ascontiguousarray(Wqkv[:, 2 * C:3 * C][:, sl]),
            "bq": np.ascontiguousarray(bqkv[0 * C:1 * C][sl]),
            "bk": np.ascontiguousarray(bqkv[1 * C:2 * C][sl]),
            "bv": np.ascontiguousarray(bqkv[2 * C:3 * C][sl]),
            "wp": np.ascontiguousarray(Wproj[g * GC:(g + 1) * GC, :]),
            "pad": padding_mask[b].astype(np.float32),
            "mask": masks,
        })

    trace = bool(os.environ.get("BASS_KERNEL_TRACE"))
    res = run_bass_kernel_spmd(
        nc, in_maps, core_ids=list(range(N_CORES)), trace=trace)
    _cached["last_result"] = res

    out = np.empty((B, T, C), dtype=np.float32)
    for b in range(B):
        out[b] = res.results[2 * b]["out"] + res.results[2 * b + 1]["out"] + bproj
    return out


# revision 6
# speedup vs baseline: 1.6578x; 1.6578x over previous
"""Causal self-attention (B=4, T=2048, C=1024, H=16) on 8 Trainium2 NeuronCores.

Sharding: core = (batch b = core//2, head-group g = core%2, 8 heads each).
Per core:
  - QKV projection for its 512 q/k/v channels: fp32r matmuls (full PE rate).
    The 1/sqrt(D) scale and biases fold into the PSUM evacuation (DVE
    tensor_scalar), reserving ScalarE for exp.
  - Attention per head pair: S^T = K^T.T @ Q^T row-tiled (the two heads use
    disjoint 64-row groups of the PE array and run concurrently), one exp per
    k-tile over a merged 2-bank PSUM tile, causal tril mask multiplied on DVE
    for diagonal tiles only (above-diagonal tiles skipped; diagonal tiles are
    restricted to their valid q-range), PV matmul accumulates with a
    ones*pad column appended to V' so row 64 of the accumulator is the
    softmax denominator (padding mask folded into V' at zero per-tile cost).
    The k-loop is software pipelined: S/exp run one step ahead of PV.
  - Normalize: DVE copy (partition crossbar 64->0), reciprocal_approx_fast,
    gpsimd partition-broadcast, DVE multiply into y^T.
  - Output projection rows slice -> partial [T, C] output.
Host: transposes x per batch, slices Wqkv/Wproj by head group, sums the two
partials per batch and adds bproj.
"""

import os
import sys

for _p in ("/opt/trn_rl_repo",):
    if _p not in sys.path:
        sys.path.append(_p)

import numpy as np

B, T, C = 4, 2048, 1024
H, D = 16, 64
HPC = 8          # heads per core
GC = HPC * D     # 512 channels per core
N_CORES = 8
P = 128
NT = T // 512    # 4  q-tiles / n-slices of 512
MT = GC // 128   # 4  m-tiles (head pairs)
CT = C // 128    # 8  contraction tiles
TT = T // 128    # 16 t-tiles of 128

_cached = {}


def _build():
    import concourse.tile as tile
    from concourse import bacc, mybir
    import concourse.bass as bass

    f32 = mybir.dt.float32
    f32r = mybir.dt.float32r
    AF = mybir.ActivationFunctionType
    ADD = mybir.AluOpType.add
    MUL = mybir.AluOpType.mult

    nc = bacc.Bacc("TRN2", target_bir_lowering=False, debug=False)

    xT_d = nc.dram_tensor("xT", [C, T], f32, kind="ExternalInput")
    wq_d = nc.dram_tensor("wq", [C, GC], f32, kind="ExternalInput")
    wk_d = nc.dram_tensor("wk", [C, GC], f32, kind="ExternalInput")
    wv_d = nc.dram_tensor("wv", [C, GC], f32, kind="ExternalInput")
    bq_d = nc.dram_tensor("bq", [GC], f32, kind="ExternalInput")
    bk_d = nc.dram_tensor("bk", [GC], f32, kind="ExternalInput")
    bv_d = nc.dram_tensor("bv", [GC], f32, kind="ExternalInput")
    wp_d = nc.dram_tensor("wp", [GC, C], f32, kind="ExternalInput")
    pad_d = nc.dram_tensor("pad", [T], f32, kind="ExternalInput")
    mask_d = nc.dram_tensor("mask", [P, 512], f32, kind="ExternalInput")
    out_d = nc.dram_tensor("out", [T, C], f32, kind="ExternalOutput")

    with tile.TileContext(nc) as tc:
        with tc.tile_pool(name="persist", bufs=1) as persist, \
             tc.tile_pool(name="allps", bufs=2, space="PSUM") as allps:
            QT = persist.tile([P, MT, T], f32r, tag="QT")
            KT = persist.tile([P, MT, T], f32r, tag="KT")
            Vp = persist.tile([P, TT, HPC, D + 1], f32r, tag="Vp")
            pad_s = persist.tile([P, TT], f32, tag="pad")
            bq_s = persist.tile([P, MT], f32, tag="bq")
            bk_s = persist.tile([P, MT], f32, tag="bk")
            bv_s = persist.tile([P, GC], f32, tag="bv")
            tril_s = persist.tile([P, 512], f32r, tag="tril")

            nc.sync.dma_start(pad_s[:], pad_d.rearrange("(tt p) -> p tt", p=P))
            nc.sync.dma_start(bq_s[:], bq_d.rearrange("(m p) -> p m", p=P))
            nc.sync.dma_start(bk_s[:], bk_d.rearrange("(m p) -> p m", p=P))
            bv_ap = bass.AP(tensor=bv_d[:].tensor, offset=0, ap=[[0, P], [1, GC]])
            nc.sync.dma_start(bv_s[:], bv_ap)
            nc.sync.dma_start(tril_s[:], mask_d[:].bitcast(f32r))
            # Vp pad column: Vp[:, tt, h, 64] = pad[tt*128 + p] for all h
            for tt in range(TT):
                nc.vector.memset(Vp[:, tt, :, D:D + 1].bitcast(f32), 1.0)
                nc.vector.tensor_scalar(
                    out=Vp[:, tt, :, D:D + 1], in0=Vp[:, tt, :, D:D + 1],
                    scalar1=pad_s[:, tt:tt + 1], scalar2=None, op0=MUL)

            xTr = xT_d.rearrange("(c p) t -> p c t", p=P).bitcast(f32r)

            # ---------------- phase 1: V + Q^T + K^T projections ----------
            with tc.tile_pool(name="wpool", bufs=1) as wpool, \
                 tc.tile_pool(name="xpool", bufs=2) as xpool, \
                 tc.tile_pool(name="tpool", bufs=2) as tpool:
                # first x slice before the weights so PE work starts early
                xtn0 = xpool.tile([P, CT, 512], f32r, tag="xtn")
                nc.sync.dma_start(xtn0[:], xTr[:, :, 0:512])
                wv_s = wpool.tile([P, CT, GC], f32r, tag="wv")
                wq_s = wpool.tile([P, CT, GC], f32r, tag="wq")
                wk_s = wpool.tile([P, CT, GC], f32r, tag="wk")
                nc.sync.dma_start(wv_s[:], wv_d.rearrange("(c p) n -> p c n", p=P).bitcast(f32r))
                nc.sync.dma_start(wq_s[:], wq_d.rearrange("(c p) n -> p c n", p=P).bitcast(f32r))
                nc.sync.dma_start(wk_s[:], wk_d.rearrange("(c p) n -> p c n", p=P).bitcast(f32r))

                for nt in range(NT):
                    if nt == 0:
                        xt_n = xtn0
                    else:
                        xt_n = xpool.tile([P, CT, 512], f32r, tag="xtn")
                        nc.sync.dma_start(xt_n[:], xTr[:, :, nt * 512:(nt + 1) * 512])
                    # V for 4 t-subtiles of this n-slice
                    for ts in range(4):
                        tt = nt * 4 + ts
                        ps = allps.tile([P, GC], f32, tag="SS")
                        for c in range(CT):
                            nc.tensor.matmul(
                                ps[:], xt_n[:, c, ts * P:(ts + 1) * P], wv_s[:, c, :],
                                start=(c == 0), stop=(c == CT - 1))
                        tmp = tpool.tile([P, GC], f32, tag="vtmp")
                        nc.vector.tensor_add(tmp[:], ps[:], bv_s[:])
                        nc.vector.tensor_scalar(
                            out=Vp[:, tt, :, 0:D],
                            in0=tmp[:].rearrange("p (h d) -> p h d", h=HPC),
                            scalar1=pad_s[:, tt:tt + 1], scalar2=None, op0=MUL)
                    # Q^T and K^T m-tiles for this n-slice
                    for W, bias, OUT, qscale in (
                            (wq_s, bq_s, QT, True), (wk_s, bk_s, KT, False)):
                        for m in range(MT):
                            ps = allps.tile([P, 512], f32, tag="SS")
                            for c in range(CT):
                                nc.tensor.matmul(
                                    ps[:], W[:, c, m * P:(m + 1) * P], xt_n[:, c, :],
                                    start=(c == 0), stop=(c == CT - 1))
                            if qscale:
                                nc.vector.tensor_scalar(
                                    out=OUT[:, m, nt * 512:(nt + 1) * 512], in0=ps[:],
                                    scalar1=bias[:, m:m + 1], scalar2=0.125,
                                    op0=ADD, op1=MUL)
                            else:
                                nc.vector.tensor_scalar(
                                    out=OUT[:, m, nt * 512:(nt + 1) * 512], in0=ps[:],
                                    scalar1=bias[:, m:m + 1], scalar2=None, op0=ADD)

            # ---------------- phase 2: attention + projection -------------
            with tc.tile_pool(name="apool", bufs=1) as apool, \
                 tc.tile_pool(name="ypool", bufs=2) as ypool, \
                 tc.tile_pool(name="ppool", bufs=4) as ppool, \
                 tc.tile_pool(name="bpool", bufs=2) as bpool, \
                 tc.tile_pool(name="prpool", bufs=3) as prpool:
                wp_s = apool.tile([P, MT, C], f32r, tag="wp")
                nc.sync.dma_start(wp_s[:], wp_d.rearrange("(m p) n -> p m n", p=P).bitcast(f32r))

                for qt in range(NT):
                    yTq = ypool.tile([P, MT, 512], f32r, tag="yTq")
                    nk = 4 * (qt + 1)
                    for j in range(MT):
                        OO = allps.tile([D + 1, 2, 512], f32, tag="OO")
                        pend = None
                        for kt in range(nk):
                            off = kt - 4 * qt
                            q0 = max(off, 0) * P
                            SS = allps.tile([P, 2, 512], f32, tag="SS")
                            nc.tensor.matmul(
                                SS[:, 0, q0:512], KT[0:D, j, kt * P:(kt + 1) * P],
                                QT[0:D, j, qt * 512 + q0:(qt + 1) * 512],
                                start=True, stop=True)
                            nc.tensor.matmul(
                                SS[:, 1, q0:512], KT[D:P, j, kt * P:(kt + 1) * P],
                                QT[D:P, j, qt * 512 + q0:(qt + 1) * 512],
                                start=True, stop=True)
                            PP = ppool.tile([P, 2, 512], f32r, tag="PP")
                            nc.scalar.activation(
                                PP[:, :, q0:512], SS[:, :, q0:512], AF.Exp)
                            if off >= 0:
                                # causal prefix of the tril mask, bcast over heads
                                tm = tril_s[:, 0:512 - q0]
                                mask_b = bass.AP(
                                    tensor=tm.tensor, offset=tm.offset,
                                    ap=[list(tm.ap[0]), [0, 2], list(tm.ap[1])])
                                nc.vector.tensor_mul(
                                    PP[:, :, q0:512], PP[:, :, q0:512], mask_b)
                            if pend is not None:
                                k_, z_, PP_ = pend
                                for e in range(2):
                                    nc.tensor.matmul(
                                        OO[:, e, z_:512], Vp[:, k_, 2 * j + e, :],
                                        PP_[:, e, z_:512],
                                        start=(k_ == 0), stop=False)
                            pend = (kt, q0, PP)
                        k_, z_, PP_ = pend
                        for e in range(2):
                            nc.tensor.matmul(
                                OO[:, e, z_:512], Vp[:, k_, 2 * j + e, :],
                                PP_[:, e, z_:512],
                                start=(k_ == 0), stop=True)
                        # normalize: l rows -> partition 0, recip, bcast, mul
                        lraw = bpool.tile([1, 2, 512], f32, tag="lraw")
                        lrec = bpool.tile([1, 2, 512], f32, tag="lrec")
                        nc.vector.tensor_copy(lraw[0:1, :, :], OO[D:D + 1, :, :])
                        nc.vector.reciprocal_approx_fast(lrec[0:1, :, :], lraw[0:1, :, :])
                        bc = bpool.tile([P, 2, 512], f32, tag="bc")
                        nc.gpsimd.partition_broadcast(bc[:], lrec[0:1, :, :], channels=P)
                        nc.vector.tensor_mul(yTq[0:D, j, :], OO[0:D, 0, :], bc[0:D, 0, :])
                        nc.vector.tensor_mul(yTq[D:P, j, :], OO[0:D, 1, :], bc[D:P, 1, :])
                    # projection for this q block of 512 rows
                    for ts in range(4):
                        tt = qt * 4 + ts
                        for nh in range(2):
                            ps = allps.tile([P, 512], f32, tag="OO")
                            for cj in range(MT):
                                nc.tensor.matmul(
                                    ps[:], yTq[:, cj, ts * P:(ts + 1) * P],
                                    wp_s[:, cj, nh * 512:(nh + 1) * 512],
                                    start=(cj == 0), stop=(cj == MT - 1))
                            ot = prpool.tile([P, 512], f32, tag="ot")
                            nc.scalar.copy(ot[:], ps[:])
                            nc.sync.dma_start(
                                out_d[tt * P:(tt + 1) * P, nh * 512:(nh + 1) * 512], ot[:])

    nc.compile()
    return nc


def _get_nc():
    if "nc" not in _cached:
        _cached["nc"] = _build()
    return _cached["nc"]


def kernel(x, padding_mask, Wqkv, bqkv, Wproj, bproj):
    from concourse.bass_utils import run_bass_kernel_spmd

    x = np.asarray(x, dtype=np.float32)
    padding_mask = np.asarray(padding_mask)
    Wqkv = np.asarray(Wqkv, dtype=np.float32)
    bqkv = np.asarray(bqkv, dtype=np.float32)
    Wproj = np.asarray(Wproj, dtype=np.float32)
    bproj = np.asarray(bproj, dtype=np.float32)
    assert x.shape == (B, T, C), x.shape

    nc = _get_nc()
    kk = np.arange(P)[:, None]
    qq = np.arange(512)[None, :]
    tril = (kk <= qq).astype(np.float32)

    in_maps = []
    for core in range(N_CORES):
        b, g = divmod(core, 2)
        sl = slice(g * GC, (g + 1) * GC)
        in_maps.append({
            "xT": np.ascontiguousarray(x[b].T),
            "wq": np.ascontiguousarray(Wqkv[:, 0 * C:1 * C][:, sl]),
            "wk": np.ascontiguousarray(Wqkv[:, 1 * C:2 * C][:, sl]),
            "wv": np.ascontiguousarray(Wqkv[:, 2 * C:3 * C][:, sl]),
            "bq": np.ascontiguousarray(bqkv[0 * C:1 * C][sl]),
            "bk": np.ascontiguousarray(bqkv[1 * C:2 * C][sl]),
            "bv": np.ascontiguousarray(bqkv[2 * C:3 * C][sl]),
            "wp": np.ascontiguousarray(Wproj[g * GC:(g + 1) * GC, :]),
            "pad": padding_mask[b].astype(np.float32),
            "mask": tril,
        })

    trace = bool(os.environ.get("BASS_KERNEL_TRACE"))
    res = run_bass_kernel_spmd(
        nc, in_maps, core_ids=list(range(N_CORES)), trace=trace)
    _cached["last_result"] = res

    out = np.empty((B, T, C), dtype=np.float32)
    for b in range(B):
        out[b] = res.results[2 * b]["out"] + res.results[2 * b + 1]["out"] + bproj
    return out


# revision 8
# speedup vs baseline: 1.7446x; 1.0524x over previous
"""Causal self-attention (B=4, T=2048, C=1024, H=16) on 8 Trainium2 NeuronCores.

Sharding: core = (batch b = core//2, head-group g = core%2, 8 heads each).
Per core:
  - QKV projection for its 512 q/k/v channels: fp32r matmuls (full PE rate).
    The 1/sqrt(D) scale and biases fold into the PSUM evacuation (DVE
    tensor_scalar), reserving ScalarE for exp.
  - Attention per head pair: S^T = K^T.T @ Q^T row-tiled (the two heads use
    disjoint 64-row groups of the PE array and run concurrently), one exp per
    k-tile over a merged 2-bank PSUM tile, causal tril mask multiplied on DVE
    for diagonal tiles only (above-diagonal tiles skipped; diagonal tiles are
    restricted to their valid q-range), PV matmul accumulates with a
    ones*pad column appended to V' so row 64 of the accumulator is the
    softmax denominator (padding mask folded into V' at zero per-tile cost).
    The k-loop is software pipelined: S/exp run one step ahead of PV.
  - Normalize: DVE copy (partition crossbar 64->0), reciprocal_approx_fast,
    gpsimd partition-broadcast, DVE multiply into y^T.
  - Output projection rows slice -> partial [T, C] output.
Host: transposes x per batch, slices Wqkv/Wproj by head group, sums the two
partials per batch and adds bproj.
"""

import os
import sys

for _p in ("/opt/trn_rl_repo",):
    if _p not in sys.path:
        sys.path.append(_p)

import numpy as np

B, T, C = 4, 2048, 1024
H, D = 16, 64
HPC = 8          # heads per core
GC = HPC * D     # 512 channels per core
N_CORES = 8
P = 128
NT = T // 512    # 4  q-tiles / n-slices of 512
MT = GC // 128   # 4  m-tiles (head pairs)
CT = C // 128    # 8  contraction tiles
TT = T // 128    # 16 t-tiles of 128

_cached = {}


def _build():
    import concourse.tile as tile
    from concourse import bacc, mybir
    import concourse.bass as bass

    f32 = mybir.dt.float32
    f32r = mybir.dt.float32r
    AF = mybir.ActivationFunctionType
    ADD = mybir.AluOpType.add
    MUL = mybir.AluOpType.mult

    nc = bacc.Bacc("TRN2", target_bir_lowering=False, debug=False)

    xT_d = nc.dram_tensor("xT", [C, T], f32, kind="ExternalInput")
    wq_d = nc.dram_tensor("wq", [C, GC], f32, kind="ExternalInput")
    wk_d = nc.dram_tensor("wk", [C, GC], f32, kind="ExternalInput")
    wv_d = nc.dram_tensor("wv", [C, GC], f32, kind="ExternalInput")
    bq_d = nc.dram_tensor("bq", [GC], f32, kind="ExternalInput")
    bk_d = nc.dram_tensor("bk", [GC], f32, kind="ExternalInput")
    bv_d = nc.dram_tensor("bv", [GC], f32, kind="ExternalInput")
    wp_d = nc.dram_tensor("wp", [GC, C], f32, kind="ExternalInput")
    pad_d = nc.dram_tensor("pad", [T], f32, kind="ExternalInput")
    mask_d = nc.dram_tensor("mask", [P, 512], f32, kind="ExternalInput")
    out_d = nc.dram_tensor("out", [T, C], f32, kind="ExternalOutput")

    with tile.TileContext(nc) as tc:
        with tc.tile_pool(name="persist", bufs=1) as persist, \
             tc.tile_pool(name="allps", bufs=2, space="PSUM") as allps:
            QT = persist.tile([P, MT, T], f32r, tag="QT")
            KT = persist.tile([P, MT, T], f32r, tag="KT")
            Vp = persist.tile([P, TT, HPC, D + 1], f32r, tag="Vp")
            pad_s = persist.tile([P, TT], f32, tag="pad")
            bq_s = persist.tile([P, MT], f32, tag="bq")
            bk_s = persist.tile([P, MT], f32, tag="bk")
            bv_s = persist.tile([P, GC], f32, tag="bv")
            tril_s = persist.tile([P, 512], f32r, tag="tril")

            nc.sync.dma_start(pad_s[:], pad_d.rearrange("(tt p) -> p tt", p=P))
            nc.sync.dma_start(bq_s[:], bq_d.rearrange("(m p) -> p m", p=P))
            nc.sync.dma_start(bk_s[:], bk_d.rearrange("(m p) -> p m", p=P))
            bv_ap = bass.AP(tensor=bv_d[:].tensor, offset=0, ap=[[0, P], [1, GC]])
            nc.sync.dma_start(bv_s[:], bv_ap)
            nc.sync.dma_start(tril_s[:], mask_d[:].bitcast(f32r))
            # Vp pad column: Vp[:, tt, h, 64] = pad[tt*128 + p] for all h
            for tt in range(TT):
                nc.vector.memset(Vp[:, tt, :, D:D + 1].bitcast(f32), 1.0)
                nc.vector.tensor_scalar(
                    out=Vp[:, tt, :, D:D + 1], in0=Vp[:, tt, :, D:D + 1],
                    scalar1=pad_s[:, tt:tt + 1], scalar2=None, op0=MUL)

            xTr = xT_d.rearrange("(c p) t -> p c t", p=P).bitcast(f32r)

            # ---------------- phase 1: V + Q^T + K^T projections ----------
            with tc.tile_pool(name="wpool", bufs=1) as wpool, \
                 tc.tile_pool(name="xpool", bufs=2) as xpool, \
                 tc.tile_pool(name="tpool", bufs=2) as tpool:
                # first x slice before the weights so PE work starts early
                xtn0 = xpool.tile([P, CT, 512], f32r, tag="xtn")
                nc.sync.dma_start(xtn0[:], xTr[:, :, 0:512])
                wv_s = wpool.tile([P, CT, GC], f32r, tag="wv")
                wq_s = wpool.tile([P, CT, GC], f32r, tag="wq")
                wk_s = wpool.tile([P, CT, GC], f32r, tag="wk")
                nc.sync.dma_start(wv_s[:], wv_d.rearrange("(c p) n -> p c n", p=P).bitcast(f32r))
                nc.sync.dma_start(wq_s[:], wq_d.rearrange("(c p) n -> p c n", p=P).bitcast(f32r))
                nc.sync.dma_start(wk_s[:], wk_d.rearrange("(c p) n -> p c n", p=P).bitcast(f32r))

                for nt in range(NT):
                    if nt == 0:
                        xt_n = xtn0
                    else:
                        xt_n = xpool.tile([P, CT, 512], f32r, tag="xtn")
                        nc.sync.dma_start(xt_n[:], xTr[:, :, nt * 512:(nt + 1) * 512])
                    # V for 4 t-subtiles of this n-slice
                    for ts in range(4):
                        tt = nt * 4 + ts
                        ps = allps.tile([P, GC], f32, tag="SS")
                        for c in range(CT):
                            nc.tensor.matmul(
                                ps[:], xt_n[:, c, ts * P:(ts + 1) * P], wv_s[:, c, :],
                                start=(c == 0), stop=(c == CT - 1))
                        tmp = tpool.tile([P, GC], f32, tag="vtmp")
                        nc.vector.tensor_add(tmp[:], ps[:], bv_s[:])
                        nc.vector.tensor_scalar(
                            out=Vp[:, tt, :, 0:D],
                            in0=tmp[:].rearrange("p (h d) -> p h d", h=HPC),
                            scalar1=pad_s[:, tt:tt + 1], scalar2=None, op0=MUL)
                    # Q^T and K^T m-tiles for this n-slice
                    for W, bias, OUT, qscale in (
                            (wq_s, bq_s, QT, True), (wk_s, bk_s, KT, False)):
                        for m in range(MT):
                            ps = allps.tile([P, 512], f32, tag="SS")
                            for c in range(CT):
                                nc.tensor.matmul(
                                    ps[:], W[:, c, m * P:(m + 1) * P], xt_n[:, c, :],
                                    start=(c == 0), stop=(c == CT - 1))
                            if qscale:
                                nc.vector.tensor_scalar(
                                    out=OUT[:, m, nt * 512:(nt + 1) * 512], in0=ps[:],
                                    scalar1=bias[:, m:m + 1], scalar2=0.125,
                                    op0=ADD, op1=MUL)
                            else:
                                nc.vector.tensor_scalar(
                                    out=OUT[:, m, nt * 512:(nt + 1) * 512], in0=ps[:],
                                    scalar1=bias[:, m:m + 1], scalar2=None, op0=ADD)

            # ---------------- phase 2: attention + projection -------------
            with tc.tile_pool(name="apool", bufs=1) as apool, \
                 tc.tile_pool(name="ypool", bufs=2) as ypool, \
                 tc.tile_pool(name="ppool", bufs=5) as ppool, \
                 tc.tile_pool(name="bpool", bufs=2) as bpool, \
                 tc.tile_pool(name="prpool", bufs=3) as prpool:
                wp_s = apool.tile([P, MT, C], f32r, tag="wp")
                nc.sync.dma_start(wp_s[:], wp_d.rearrange("(m p) n -> p m n", p=P).bitcast(f32r))

                for qt in range(NT):
                    yTq = ypool.tile([P, MT, 512], f32r, tag="yTq")
                    nk = 4 * (qt + 1)
                    for j in range(MT):
                        OO = allps.tile([D + 1, 2, 512], f32, tag="OO")
                        pend = []  # PV lags S/exp by LAG k-tiles
                        LAG = 2

                        def emit_pv(entry, last):
                            k_, z_, PP_ = entry
                            for e in range(2):
                                nc.tensor.matmul(
                                    OO[:, e, z_:512], Vp[:, k_, 2 * j + e, :],
                                    PP_[:, e, z_:512],
                                    start=(k_ == 0), stop=last)

                        for kt in range(nk):
                            off = kt - 4 * qt
                            q0 = max(off, 0) * P
                            SS = allps.tile([P, 2, 512], f32, tag="SS")
                            nc.tensor.matmul(
                                SS[:, 0, q0:512], KT[0:D, j, kt * P:(kt + 1) * P],
                                QT[0:D, j, qt * 512 + q0:(qt + 1) * 512],
                                start=True, stop=True)
                            nc.tensor.matmul(
                                SS[:, 1, q0:512], KT[D:P, j, kt * P:(kt + 1) * P],
                                QT[D:P, j, qt * 512 + q0:(qt + 1) * 512],
                                start=True, stop=True)
                            PP = ppool.tile([P, 2, 512], f32r, tag="PP")
                            nc.scalar.activation(
                                PP[:, :, q0:512], SS[:, :, q0:512], AF.Exp)
                            if off >= 0:
                                # causal prefix of the tril mask, bcast over heads
                                tm = tril_s[:, 0:512 - q0]
                                mask_b = bass.AP(
                                    tensor=tm.tensor, offset=tm.offset,
                                    ap=[list(tm.ap[0]), [0, 2], list(tm.ap[1])])
                                nc.vector.tensor_mul(
                                    PP[:, :, q0:512], PP[:, :, q0:512], mask_b)
                            pend.append((kt, q0, PP))
                            if len(pend) > LAG:
                                emit_pv(pend.pop(0), False)
                        while pend:
                            emit_pv(pend.pop(0), len(pend) == 0)
                        # normalize: l rows -> partition 0, recip, bcast, mul
                        lraw = bpool.tile([1, 2, 512], f32, tag="lraw")
                        lrec = bpool.tile([1, 2, 512], f32, tag="lrec")
                        nc.vector.tensor_copy(lraw[0:1, :, :], OO[D:D + 1, :, :])
                        nc.vector.reciprocal_approx_fast(lrec[0:1, :, :], lraw[0:1, :, :])
                        bc = bpool.tile([P, 2, 512], f32, tag="bc")
                        nc.gpsimd.partition_broadcast(bc[:], lrec[0:1, :, :], channels=P)
                        nc.vector.tensor_mul(yTq[0:D, j, :], OO[0:D, 0, :], bc[0:D, 0, :])
                        nc.vector.tensor_mul(yTq[D:P, j, :], OO[0:D, 1, :], bc[D:P, 1, :])
                    # projection for this q block of 512 rows
                    for ts in range(4):
                        tt = qt * 4 + ts
                        for nh in range(2):
                            ps = allps.tile([P, 512], f32, tag="OO")
                            for cj in range(MT):
                                nc.tensor.matmul(
                                    ps[:], yTq[:, cj, ts * P:(ts + 1) * P],
                                    wp_s[:, cj, nh * 512:(nh + 1) * 512],
                                    start=(cj == 0), stop=(cj == MT - 1))
                            ot = prpool.tile([P, 512], f32, tag="ot")
                            nc.vector.tensor_copy(ot[:], ps[:])
                            nc.sync.dma_start(
                                out_d[tt * P:(tt + 1) * P, nh * 512:(nh + 1) * 512], ot[:])

    nc.compile()
    return nc


def _get_nc():
    if "nc" not in _cached:
        _cached["nc"] = _build()
    return _cached["nc"]


def kernel(x, padding_mask, Wqkv, bqkv, Wproj, bproj):
    from concourse.bass_utils import run_bass_kernel_spmd

    x = np.asarray(x, dtype=np.float32)
    padding_mask = np.asarray(padding_mask)
    Wqkv = np.asarray(Wqkv, dtype=np.float32)
    bqkv = np.asarray(bqkv, dtype=np.float32)
    Wproj = np.asarray(Wproj, dtype=np.float32)
    bproj = np.asarray(bproj, dtype=np.float32)
    assert x.shape == (B, T, C), x.shape

    nc = _get_nc()
    kk = np.arange(P)[:, None]
    qq = np.arange(512)[None, :]
    tril = (kk <= qq).astype(np.float32)

    in_maps = []
    for core in range(N_CORES):
        b, g = divmod(core, 2)
        sl = slice(g * GC, (g + 1) * GC)
        in_maps.append({
            "xT": np.ascontiguousarray(x[b].T),
            "wq": np.ascontiguousarray(Wqkv[:, 0 * C:1 * C][:, sl]),
            "wk": np.ascontiguousarray(Wqkv[:, 1 * C:2 * C][:, sl]),
            "wv": np.ascontiguousarray(Wqkv[:, 2 * C:3 * C][:, sl]),
            "bq": np.ascontiguousarray(bqkv[0 * C:1 * C][sl]),
            "bk": np.ascontiguousarray(bqkv[1 * C:2 * C][sl]),
            "bv": np.ascontiguousarray(bqkv[2 * C:3 * C][sl]),
            "wp": np.ascontiguousarray(Wproj[g * GC:(g + 1) * GC, :]),
            "pad": padding_mask[b].astype(np.float32),
            "mask": tril,
        })

    trace = bool(os.environ.get("BASS_KERNEL_TRACE"))
    res = run_bass_kernel_spmd(
        nc, in_maps, core_ids=list(range(N_CORES)), trace=trace)
    _cached["last_result"] = res

    out = np.empty((B, T, C), dtype=np.float32)
    for b in range(B):
        out[b] = res.results[2 * b]["out"] + res.results[2 * b + 1]["out"] + bproj
    return out


# revision 9
# speedup vs baseline: 1.7517x; 1.0040x over previous
"""Causal self-attention (B=4, T=2048, C=1024, H=16) on 8 Trainium2 NeuronCores.

Sharding: core = (batch b = core//2, head-group g = core%2, 8 heads each).
Per core:
  - QKV projection for its 512 q/k/v channels: fp32r matmuls (full PE rate).
    The 1/sqrt(D) scale and biases fold into the PSUM evacuation (DVE
    tensor_scalar), reserving ScalarE for exp.
  - Attention per head pair: S^T = K^T.T @ Q^T row-tiled (the two heads use
    disjoint 64-row groups of the PE array and run concurrently), one exp per
    k-tile over a merged 2-bank PSUM tile, causal tril mask multiplied on DVE
    for diagonal tiles only (above-diagonal tiles skipped; diagonal tiles are
    restricted to their valid q-range), PV matmul accumulates with a
    ones*pad column appended to V' so row 64 of the accumulator is the
    softmax denominator (padding mask folded into V' at zero per-tile cost).
    The k-loop is software pipelined: S/exp run one step ahead of PV.
  - Normalize: DVE copy (partition crossbar 64->0), reciprocal_approx_fast,
    gpsimd partition-broadcast, DVE multiply into y^T.
  - Output projection rows slice -> partial [T, C] output.
Host: transposes x per batch, slices Wqkv/Wproj by head group, sums the two
partials per batch and adds bproj.
"""

import os
import sys

for _p in ("/opt/trn_rl_repo",):
    if _p not in sys.path:
        sys.path.append(_p)

import numpy as np

B, T, C = 4, 2048, 1024
H, D = 16, 64
HPC = 8          # heads per core
GC = HPC * D     # 512 channels per core
N_CORES = 8
P = 128
NT = T // 512    # 4  q-tiles / n-slices of 512
MT = GC // 128   # 4  m-tiles (head pairs)
CT = C // 128    # 8  contraction tiles
TT = T // 128    # 16 t-tiles of 128

_cached = {}


def _build():
    import concourse.tile as tile
    from concourse import bacc, mybir
    import concourse.bass as bass

    f32 = mybir.dt.float32
    f32r = mybir.dt.float32r
    AF = mybir.ActivationFunctionType
    ADD = mybir.AluOpType.add
    MUL = mybir.AluOpType.mult

    nc = bacc.Bacc("TRN2", target_bir_lowering=False, debug=False)

    xT_d = nc.dram_tensor("xT", [C, T], f32, kind="ExternalInput")
    wq_d = nc.dram_tensor("wq", [C, GC], f32, kind="ExternalInput")
    wk_d = nc.dram_tensor("wk", [C, GC], f32, kind="ExternalInput")
    wv_d = nc.dram_tensor("wv", [C, GC], f32, kind="ExternalInput")
    bq_d = nc.dram_tensor("bq", [GC], f32, kind="ExternalInput")
    bk_d = nc.dram_tensor("bk", [GC], f32, kind="ExternalInput")
    bv_d = nc.dram_tensor("bv", [GC], f32, kind="ExternalInput")
    wp_d = nc.dram_tensor("wp", [GC, C], f32, kind="ExternalInput")
    pad_d = nc.dram_tensor("pad", [T], f32, kind="ExternalInput")
    mask_d = nc.dram_tensor("mask", [P, 512], f32, kind="ExternalInput")
    out_d = nc.dram_tensor("out", [T, C], f32, kind="ExternalOutput")

    with tile.TileContext(nc) as tc:
        with tc.tile_pool(name="persist", bufs=1) as persist, \
             tc.tile_pool(name="allps", bufs=2, space="PSUM") as allps:
            QT = persist.tile([P, MT, T], f32r, tag="QT")
            KT = persist.tile([P, MT, T], f32r, tag="KT")
            Vp = persist.tile([P, TT, HPC, D + 1], f32r, tag="Vp")
            pad_s = persist.tile([P, TT], f32, tag="pad")
            bq_s = persist.tile([P, MT], f32, tag="bq")
            bk_s = persist.tile([P, MT], f32, tag="bk")
            bv_s = persist.tile([P, GC], f32, tag="bv")
            tril_s = persist.tile([P, 512], f32r, tag="tril")

            nc.sync.dma_start(pad_s[:], pad_d.rearrange("(tt p) -> p tt", p=P))
            nc.sync.dma_start(bq_s[:], bq_d.rearrange("(m p) -> p m", p=P))
            nc.sync.dma_start(bk_s[:], bk_d.rearrange("(m p) -> p m", p=P))
            bv_ap = bass.AP(tensor=bv_d[:].tensor, offset=0, ap=[[0, P], [1, GC]])
            nc.sync.dma_start(bv_s[:], bv_ap)
            nc.sync.dma_start(tril_s[:], mask_d[:].bitcast(f32r))
            # Vp pad column: Vp[:, tt, h, 64] = pad[tt*128 + p] for all h
            for tt in range(TT):
                nc.vector.memset(Vp[:, tt, :, D:D + 1].bitcast(f32), 1.0)
                nc.vector.tensor_scalar(
                    out=Vp[:, tt, :, D:D + 1], in0=Vp[:, tt, :, D:D + 1],
                    scalar1=pad_s[:, tt:tt + 1], scalar2=None, op0=MUL)

            xTr = xT_d.rearrange("(c p) t -> p c t", p=P).bitcast(f32r)

            # ---------------- phase 1: V + Q^T + K^T projections ----------
            with tc.tile_pool(name="wpool", bufs=1) as wpool, \
                 tc.tile_pool(name="xpool", bufs=2) as xpool, \
                 tc.tile_pool(name="tpool", bufs=2) as tpool:
                # first x slice before the weights so PE work starts early;
                # chunked DMAs so the first matmuls' inputs land quickly
                xtn0 = xpool.tile([P, CT, 512], f32r, tag="xtn")
                for c2 in range(0, CT, 2):
                    nc.sync.dma_start(
                        xtn0[:, c2:c2 + 2, :], xTr[:, c2:c2 + 2, 0:512])
                wv_s = wpool.tile([P, CT, GC], f32r, tag="wv")
                wq_s = wpool.tile([P, CT, GC], f32r, tag="wq")
                wk_s = wpool.tile([P, CT, GC], f32r, tag="wk")
                wvr = wv_d.rearrange("(c p) n -> p c n", p=P).bitcast(f32r)
                wqr = wq_d.rearrange("(c p) n -> p c n", p=P).bitcast(f32r)
                wkr = wk_d.rearrange("(c p) n -> p c n", p=P).bitcast(f32r)
                for c2 in range(0, CT, 2):
                    nc.sync.dma_start(wv_s[:, c2:c2 + 2, :], wvr[:, c2:c2 + 2, :])
                for c2 in range(0, CT, 2):
                    nc.sync.dma_start(wq_s[:, c2:c2 + 2, :], wqr[:, c2:c2 + 2, :])
                for c2 in range(0, CT, 2):
                    nc.sync.dma_start(wk_s[:, c2:c2 + 2, :], wkr[:, c2:c2 + 2, :])

                for nt in range(NT):
                    if nt == 0:
                        xt_n = xtn0
                    else:
                        xt_n = xpool.tile([P, CT, 512], f32r, tag="xtn")
                        nc.sync.dma_start(xt_n[:], xTr[:, :, nt * 512:(nt + 1) * 512])
                    # V for 4 t-subtiles of this n-slice
                    for ts in range(4):
                        tt = nt * 4 + ts
                        ps = allps.tile([P, GC], f32, tag="SS")
                        for c in range(CT):
                            nc.tensor.matmul(
                                ps[:], xt_n[:, c, ts * P:(ts + 1) * P], wv_s[:, c, :],
                                start=(c == 0), stop=(c == CT - 1))
                        tmp = tpool.tile([P, GC], f32, tag="vtmp")
                        nc.vector.tensor_add(tmp[:], ps[:], bv_s[:])
                        nc.vector.tensor_scalar(
                            out=Vp[:, tt, :, 0:D],
                            in0=tmp[:].rearrange("p (h d) -> p h d", h=HPC),
                            scalar1=pad_s[:, tt:tt + 1], scalar2=None, op0=MUL)
                    # Q^T and K^T m-tiles for this n-slice
                    for W, bias, OUT, qscale in (
                            (wq_s, bq_s, QT, True), (wk_s, bk_s, KT, False)):
                        for m in range(MT):
                            ps = allps.tile([P, 512], f32, tag="SS")
                            for c in range(CT):
                                nc.tensor.matmul(
                                    ps[:], W[:, c, m * P:(m + 1) * P], xt_n[:, c, :],
                                    start=(c == 0), stop=(c == CT - 1))
                            if qscale:
                                nc.vector.tensor_scalar(
                                    out=OUT[:, m, nt * 512:(nt + 1) * 512], in0=ps[:],
                                    scalar1=bias[:, m:m + 1], scalar2=0.125,
                                    op0=ADD, op1=MUL)
                            else:
                                nc.vector.tensor_scalar(
                                    out=OUT[:, m, nt * 512:(nt + 1) * 512], in0=ps[:],
                                    scalar1=bias[:, m:m + 1], scalar2=None, op0=ADD)

            # ---------------- phase 2: attention + projection -------------
            with tc.tile_pool(name="apool", bufs=1) as apool, \
                 tc.tile_pool(name="ypool", bufs=2) as ypool, \
                 tc.tile_pool(name="ppool", bufs=5) as ppool, \
                 tc.tile_pool(name="bpool", bufs=2) as bpool, \
                 tc.tile_pool(name="prpool", bufs=3) as prpool:
                wp_s = apool.tile([P, MT, C], f32r, tag="wp")
                nc.sync.dma_start(wp_s[:], wp_d.rearrange("(m p) n -> p m n", p=P).bitcast(f32r))

                for qt in range(NT):
                    yTq = ypool.tile([P, MT, 512], f32r, tag="yTq")
                    nk = 4 * (qt + 1)
                    for j in range(MT):
                        OO = allps.tile([D + 1, 2, 512], f32, tag="OO")
                        pend = []  # PV lags S/exp by LAG k-tiles
                        LAG = 2

                        def emit_pv(entry, last):
                            k_, z_, PP_ = entry
                            for e in range(2):
                                nc.tensor.matmul(
                                    OO[:, e, z_:512], Vp[:, k_, 2 * j + e, :],
                                    PP_[:, e, z_:512],
                                    start=(k_ == 0), stop=last)

                        for kt in range(nk):
                            off = kt - 4 * qt
                            q0 = max(off, 0) * P
                            SS = allps.tile([P, 2, 512], f32, tag="SS")
                            nc.tensor.matmul(
                                SS[:, 0, q0:512], KT[0:D, j, kt * P:(kt + 1) * P],
                                QT[0:D, j, qt * 512 + q0:(qt + 1) * 512],
                                start=True, stop=True)
                            nc.tensor.matmul(
                                SS[:, 1, q0:512], KT[D:P, j, kt * P:(kt + 1) * P],
                                QT[D:P, j, qt * 512 + q0:(qt + 1) * 512],
                                start=True, stop=True)
                            PP = ppool.tile([P, 2, 512], f32r, tag="PP")
                            nc.scalar.activation(
                                PP[:, :, q0:512], SS[:, :, q0:512], AF.Exp)
                            if off >= 0:
                                # causal prefix of the tril mask, bcast over heads
                                tm = tril_s[:, 0:512 - q0]
                                mask_b = bass.AP(
                                    tensor=tm.tensor, offset=tm.offset,
                                    ap=[list(tm.ap[0]), [0, 2], list(tm.ap[1])])
                                nc.vector.tensor_mul(
                                    PP[:, :, q0:512], PP[:, :, q0:512], mask_b)
                            pend.append((kt, q0, PP))
                            if len(pend) > LAG:
                                emit_pv(pend.pop(0), False)
                        while pend:
                            emit_pv(pend.pop(0), len(pend) == 0)
                        # normalize: l rows -> partition 0, recip, bcast, mul
                        lraw = bpool.tile([1, 2, 512], f32, tag="lraw")
                        lrec = bpool.tile([1, 2, 512], f32, tag="lrec")
                        nc.vector.tensor_copy(lraw[0:1, :, :], OO[D:D + 1, :, :])
                        nc.vector.reciprocal_approx_fast(lrec[0:1, :, :], lraw[0:1, :, :])
                        bc = bpool.tile([P, 2, 512], f32, tag="bc")
                        nc.gpsimd.partition_broadcast(bc[:], lrec[0:1, :, :], channels=P)
                        nc.vector.tensor_mul(yTq[0:D, j, :], OO[0:D, 0, :], bc[0:D, 0, :])
                        nc.vector.tensor_mul(yTq[D:P, j, :], OO[0:D, 1, :], bc[D:P, 1, :])
                    # projection for this q block of 512 rows
                    for ts in range(4):
                        tt = qt * 4 + ts
                        for nh in range(2):
                            ps = allps.tile([P, 512], f32, tag="OO")
                            for cj in range(MT):
                                nc.tensor.matmul(
                                    ps[:], yTq[:, cj, ts * P:(ts + 1) * P],
                                    wp_s[:, cj, nh * 512:(nh + 1) * 512],
                                    start=(cj == 0), stop=(cj == MT - 1))
                            ot = prpool.tile([P, 512], f32, tag="ot")
                            nc.vector.tensor_copy(ot[:], ps[:])
                            nc.sync.dma_start(
                                out_d[tt * P:(tt + 1) * P, nh * 512:(nh + 1) * 512], ot[:])

    nc.compile()
    return nc


def _get_nc():
    if "nc" not in _cached:
        _cached["nc"] = _build()
    return _cached["nc"]


def kernel(x, padding_mask, Wqkv, bqkv, Wproj, bproj):
    from concourse.bass_utils import run_bass_kernel_spmd

    x = np.asarray(x, dtype=np.float32)
    padding_mask = np.asarray(padding_mask)
    Wqkv = np.asarray(Wqkv, dtype=np.float32)
    bqkv = np.asarray(bqkv, dtype=np.float32)
    Wproj = np.asarray(Wproj, dtype=np.float32)
    bproj = np.asarray(bproj, dtype=np.float32)
    assert x.shape == (B, T, C), x.shape

    nc = _get_nc()
    kk = np.arange(P)[:, None]
    qq = np.arange(512)[None, :]
    tril = (kk <= qq).astype(np.float32)

    in_maps = []
    for core in range(N_CORES):
        b, g = divmod(core, 2)
        sl = slice(g * GC, (g + 1) * GC)
        in_maps.append({
            "xT": np.ascontiguousarray(x[b].T),
            "wq": np.ascontiguousarray(Wqkv[:, 0 * C:1 * C][:, sl]),
            "wk": np.ascontiguousarray(Wqkv[:, 1 * C:2 * C][:, sl]),
            "wv": np.ascontiguousarray(Wqkv[:, 2 * C:3 * C][:, sl]),
            "bq": np.ascontiguousarray(bqkv[0 * C:1 * C][sl]),
            "bk": np.ascontiguousarray(bqkv[1 * C:2 * C][sl]),
            "bv": np.ascontiguousarray(bqkv[2 * C:3 * C][sl]),
            "wp": np.ascontiguousarray(Wproj[g * GC:(g + 1) * GC, :]),
            "pad": padding_mask[b].astype(np.float32),
            "mask": tril,
        })

    trace = bool(os.environ.get("BASS_KERNEL_TRACE"))
    res = run_bass_kernel_spmd(
        nc, in_maps, core_ids=list(range(N_CORES)), trace=trace)
    _cached["last_result"] = res

    out = np.empty((B, T, C), dtype=np.float32)
    for b in range(B):
        out[b] = res.results[2 * b]["out"] + res.results[2 * b + 1]["out"] + bproj
    return out


# revision 11
# speedup vs baseline: 1.8801x; 1.0734x over previous
"""Causal self-attention (B=4, T=2048, C=1024, H=16) on 8 Trainium2 NeuronCores.

Sharding: core = (batch b = core//2, head-group g = core%2, 8 heads each).
Per core:
  - QKV projection for its 512 q/k/v channels: fp32r matmuls (full PE rate).
    The 1/sqrt(D) scale and biases fold into the PSUM evacuation (DVE
    tensor_scalar), reserving ScalarE for exp.
  - Attention per head pair: S^T = K^T.T @ Q^T row-tiled (the two heads use
    disjoint 64-row groups of the PE array and run concurrently), one exp per
    k-tile over a merged 2-bank PSUM tile, causal tril mask multiplied on DVE
    for diagonal tiles only (above-diagonal tiles skipped; diagonal tiles are
    restricted to their valid q-range), PV matmul accumulates with a
    ones*pad column appended to V' so row 64 of the accumulator is the
    softmax denominator (padding mask folded into V' at zero per-tile cost).
    The k-loop is software pipelined: S/exp run one step ahead of PV.
  - Normalize: DVE copy (partition crossbar 64->0), reciprocal_approx_fast,
    gpsimd partition-broadcast, DVE multiply into y^T.
  - Output projection rows slice -> partial [T, C] output.
Host: transposes x per batch, slices Wqkv/Wproj by head group, sums the two
partials per batch and adds bproj.
"""

import os
import sys

for _p in ("/opt/trn_rl_repo",):
    if _p not in sys.path:
        sys.path.append(_p)

import numpy as np

B, T, C = 4, 2048, 1024
H, D = 16, 64
HPC = 8          # heads per core
GC = HPC * D     # 512 channels per core
N_CORES = 8
P = 128
NT = T // 512    # 4  q-tiles / n-slices of 512
MT = GC // 128   # 4  m-tiles (head pairs)
CT = C // 128    # 8  contraction tiles
TT = T // 128    # 16 t-tiles of 128

_cached = {}


def _build():
    import concourse.tile as tile
    from concourse import bacc, mybir
    import concourse.bass as bass

    f32 = mybir.dt.float32
    f32r = mybir.dt.float32r
    AF = mybir.ActivationFunctionType
    ADD = mybir.AluOpType.add
    MUL = mybir.AluOpType.mult

    nc = bacc.Bacc("TRN2", target_bir_lowering=False, debug=False)

    xT_d = nc.dram_tensor("xT", [C, T], f32, kind="ExternalInput")
    wq_d = nc.dram_tensor("wq", [C, GC], f32, kind="ExternalInput")
    wk_d = nc.dram_tensor("wk", [C, GC], f32, kind="ExternalInput")
    wv_d = nc.dram_tensor("wv", [C, GC], f32, kind="ExternalInput")
    bq_d = nc.dram_tensor("bq", [GC], f32, kind="ExternalInput")
    bk_d = nc.dram_tensor("bk", [GC], f32, kind="ExternalInput")
    bv_d = nc.dram_tensor("bv", [GC], f32, kind="ExternalInput")
    wp_d = nc.dram_tensor("wp", [GC, C], f32, kind="ExternalInput")
    pad_d = nc.dram_tensor("pad", [T], f32, kind="ExternalInput")
    mask_d = nc.dram_tensor("mask", [P, 512], f32, kind="ExternalInput")
    out_d = nc.dram_tensor("out", [T, C], f32, kind="ExternalOutput")

    with tile.TileContext(nc) as tc:
        with tc.tile_pool(name="persist", bufs=1) as persist, \
             tc.tile_pool(name="allps", bufs=2, space="PSUM") as allps:
            QT = persist.tile([P, MT, T], f32r, tag="QT")
            KT = persist.tile([P, MT, T], f32r, tag="KT")
            Vp = persist.tile([P, TT, HPC, D + 1], f32r, tag="Vp")
            pad_s = persist.tile([P, TT], f32, tag="pad")
            bq_s = persist.tile([P, MT], f32, tag="bq")
            bk_s = persist.tile([P, MT], f32, tag="bk")
            bv_s = persist.tile([P, GC], f32, tag="bv")
            tril_s = persist.tile([P, 512], f32r, tag="tril")

            nc.sync.dma_start(pad_s[:], pad_d.rearrange("(tt p) -> p tt", p=P))
            nc.sync.dma_start(bq_s[:], bq_d.rearrange("(m p) -> p m", p=P))
            nc.sync.dma_start(bk_s[:], bk_d.rearrange("(m p) -> p m", p=P))
            bv_ap = bass.AP(tensor=bv_d[:].tensor, offset=0, ap=[[0, P], [1, GC]])
            nc.sync.dma_start(bv_s[:], bv_ap)
            nc.sync.dma_start(tril_s[:], mask_d[:].bitcast(f32r))
            # Vp pad column: Vp[:, tt, h, 64] = pad[tt*128 + p] for all h
            for tt in range(TT):
                nc.vector.memset(Vp[:, tt, :, D:D + 1].bitcast(f32), 1.0)
                nc.vector.tensor_scalar(
                    out=Vp[:, tt, :, D:D + 1], in0=Vp[:, tt, :, D:D + 1],
                    scalar1=pad_s[:, tt:tt + 1], scalar2=None, op0=MUL)

            xTr = xT_d.rearrange("(c p) t -> p c t", p=P).bitcast(f32r)

            # ---------------- phase 1: V + Q^T + K^T projections ----------
            with tc.tile_pool(name="wpool", bufs=1) as wpool, \
                 tc.tile_pool(name="xpool", bufs=2) as xpool, \
                 tc.tile_pool(name="tpool", bufs=2) as tpool:
                # first x slice before the weights so PE work starts early;
                # chunked DMAs so the first matmuls' inputs land quickly
                xtn0 = xpool.tile([P, CT, 512], f32r, tag="xtn")
                for c2 in range(0, CT, 2):
                    nc.sync.dma_start(
                        xtn0[:, c2:c2 + 2, :], xTr[:, c2:c2 + 2, 0:512])
                wv_s = wpool.tile([P, CT, GC], f32r, tag="wv")
                wq_s = wpool.tile([P, CT, GC], f32r, tag="wq")
                wk_s = wpool.tile([P, CT, GC], f32r, tag="wk")
                wvr = wv_d.rearrange("(c p) n -> p c n", p=P).bitcast(f32r)
                wqr = wq_d.rearrange("(c p) n -> p c n", p=P).bitcast(f32r)
                wkr = wk_d.rearrange("(c p) n -> p c n", p=P).bitcast(f32r)
                for c2 in range(0, CT, 2):
                    nc.sync.dma_start(wv_s[:, c2:c2 + 2, :], wvr[:, c2:c2 + 2, :])
                for c2 in range(0, CT, 2):
                    nc.sync.dma_start(wq_s[:, c2:c2 + 2, :], wqr[:, c2:c2 + 2, :])
                for c2 in range(0, CT, 2):
                    nc.sync.dma_start(wk_s[:, c2:c2 + 2, :], wkr[:, c2:c2 + 2, :])

                for nt in range(NT):
                    if nt == 0:
                        xt_n = xtn0
                    else:
                        xt_n = xpool.tile([P, CT, 512], f32r, tag="xtn")
                        nc.sync.dma_start(xt_n[:], xTr[:, :, nt * 512:(nt + 1) * 512])
                    # V for 4 t-subtiles of this n-slice
                    for ts in range(4):
                        tt = nt * 4 + ts
                        ps = allps.tile([P, GC], f32, tag="SS")
                        for c in range(CT):
                            nc.tensor.matmul(
                                ps[:], xt_n[:, c, ts * P:(ts + 1) * P], wv_s[:, c, :],
                                start=(c == 0), stop=(c == CT - 1))
                        tmp = tpool.tile([P, GC], f32, tag="vtmp")
                        nc.vector.tensor_add(tmp[:], ps[:], bv_s[:])
                        nc.vector.tensor_scalar(
                            out=Vp[:, tt, :, 0:D],
                            in0=tmp[:].rearrange("p (h d) -> p h d", h=HPC),
                            scalar1=pad_s[:, tt:tt + 1], scalar2=None, op0=MUL)
                    # Q^T and K^T m-tiles for this n-slice
                    for W, bias, OUT, qscale in (
                            (wq_s, bq_s, QT, True), (wk_s, bk_s, KT, False)):
                        for m in range(MT):
                            ps = allps.tile([P, 512], f32, tag="SS")
                            for c in range(CT):
                                nc.tensor.matmul(
                                    ps[:], W[:, c, m * P:(m + 1) * P], xt_n[:, c, :],
                                    start=(c == 0), stop=(c == CT - 1))
                            if qscale:
                                nc.vector.tensor_scalar(
                                    out=OUT[:, m, nt * 512:(nt + 1) * 512], in0=ps[:],
                                    scalar1=bias[:, m:m + 1], scalar2=0.125,
                                    op0=ADD, op1=MUL)
                            else:
                                nc.vector.tensor_scalar(
                                    out=OUT[:, m, nt * 512:(nt + 1) * 512], in0=ps[:],
                                    scalar1=bias[:, m:m + 1], scalar2=None, op0=ADD)

            # ---------------- phase 2: attention + projection -------------
            with tc.tile_pool(name="apool", bufs=1) as apool, \
                 tc.tile_pool(name="ypool", bufs=2) as ypool, \
                 tc.tile_pool(name="ppool", bufs=5) as ppool, \
                 tc.tile_pool(name="bpool", bufs=2) as bpool, \
                 tc.tile_pool(name="prpool", bufs=3) as prpool:
                wp_s = apool.tile([P, MT, C], f32r, tag="wp")
                nc.sync.dma_start(wp_s[:], wp_d.rearrange("(m p) n -> p m n", p=P).bitcast(f32r))

                def proj_piece(qt_, yT_, ts):
                    # one t-subtile (2 output halves) of the projection for
                    # q block qt_, reading the finished yT_ tile
                    tt = qt_ * 4 + ts
                    for nh in range(2):
                        ps = allps.tile([P, 512], f32, tag="OO")
                        for cj in range(MT):
                            nc.tensor.matmul(
                                ps[:], yT_[:, cj, ts * P:(ts + 1) * P],
                                wp_s[:, cj, nh * 512:(nh + 1) * 512],
                                start=(cj == 0), stop=(cj == MT - 1))
                        ot = prpool.tile([P, 512], f32, tag="ot")
                        nc.vector.tensor_copy(ot[:], ps[:])
                        nc.sync.dma_start(
                            out_d[tt * P:(tt + 1) * P, nh * 512:(nh + 1) * 512], ot[:])

                yTq_prev = None
                for qt in range(NT):
                    yTq = ypool.tile([P, MT, 512], f32r, tag="yTq")
                    nk = 4 * (qt + 1)
                    for j in range(MT):
                        OO = allps.tile([D + 1, 2, 512], f32, tag="OO")
                        pend = []  # PV lags S/exp by LAG k-tiles
                        LAG = 2

                        def emit_pv(entry, last):
                            k_, z_, PP_ = entry
                            for e in range(2):
                                nc.tensor.matmul(
                                    OO[:, e, z_:512], Vp[:, k_, 2 * j + e, :],
                                    PP_[:, e, z_:512],
                                    start=(k_ == 0), stop=last)

                        for kt in range(nk):
                            off = kt - 4 * qt
                            q0 = max(off, 0) * P
                            SS = allps.tile([P, 2, 512], f32, tag="SS")
                            nc.tensor.matmul(
                                SS[:, 0, q0:512], KT[0:D, j, kt * P:(kt + 1) * P],
                                QT[0:D, j, qt * 512 + q0:(qt + 1) * 512],
                                start=True, stop=True)
                            nc.tensor.matmul(
                                SS[:, 1, q0:512], KT[D:P, j, kt * P:(kt + 1) * P],
                                QT[D:P, j, qt * 512 + q0:(qt + 1) * 512],
                                start=True, stop=True)
                            PP = ppool.tile([P, 2, 512], f32r, tag="PP")
                            nc.scalar.activation(
                                PP[:, :, q0:512], SS[:, :, q0:512], AF.Exp)
                            if off >= 0:
                                # causal prefix of the tril mask, bcast over heads
                                tm = tril_s[:, 0:512 - q0]
                                mask_b = bass.AP(
                                    tensor=tm.tensor, offset=tm.offset,
                                    ap=[list(tm.ap[0]), [0, 2], list(tm.ap[1])])
                                nc.vector.tensor_mul(
                                    PP[:, :, q0:512], PP[:, :, q0:512], mask_b)
                            pend.append((kt, q0, PP))
                            if len(pend) > LAG:
                                emit_pv(pend.pop(0), False)
                        while pend:
                            emit_pv(pend.pop(0), len(pend) == 0)
                        # normalize: l rows -> partition 0, recip, bcast, mul
                        lraw = bpool.tile([1, 2, 512], f32, tag="lraw")
                        lrec = bpool.tile([1, 2, 512], f32, tag="lrec")
                        nc.vector.tensor_copy(lraw[0:1, :, :], OO[D:D + 1, :, :])
                        nc.vector.reciprocal_approx_fast(lrec[0:1, :, :], lraw[0:1, :, :])
                        bc = bpool.tile([P, 2, 512], f32, tag="bc")
                        nc.gpsimd.partition_broadcast(bc[:], lrec[0:1, :, :], channels=P)
                        nc.vector.tensor_mul(yTq[0:D, j, :], OO[0:D, 0, :], bc[0:D, 0, :])
                        nc.vector.tensor_mul(yTq[D:P, j, :], OO[0:D, 1, :], bc[D:P, 1, :])
                        # interleave a piece of the previous q block's
                        # projection so the PE stream never stalls on yTq
                        if yTq_prev is not None:
                            proj_piece(qt - 1, yTq_prev, j)
                    yTq_prev = yTq
                # final q block's projection
                for ts in range(4):
                    proj_piece(NT - 1, yTq_prev, ts)

    nc.compile()
    return nc


def _get_nc():
    if "nc" not in _cached:
        _cached["nc"] = _build()
    return _cached["nc"]


def kernel(x, padding_mask, Wqkv, bqkv, Wproj, bproj):
    from concourse.bass_utils import run_bass_kernel_spmd

    x = np.asarray(x, dtype=np.float32)
    padding_mask = np.asarray(padding_mask)
    Wqkv = np.asarray(Wqkv, dtype=np.float32)
    bqkv = np.asarray(bqkv, dtype=np.float32)
    Wproj = np.asarray(Wproj, dtype=np.float32)
    bproj = np.asarray(bproj, dtype=np.float32)
    assert x.shape == (B, T, C), x.shape

    nc = _get_nc()
    kk = np.arange(P)[:, None]
    qq = np.arange(512)[None, :]
    tril = (kk <= qq).astype(np.float32)

    in_maps = []
    for core in range(N_CORES):
        b, g = divmod(core, 2)
        sl = slice(g * GC, (g + 1) * GC)
        in_maps.append({
            "xT": np.ascontiguousarray(x[b].T),
            "wq": np.ascontiguousarray(Wqkv[:, 0 * C:1 * C][:, sl]),
            "wk": np.ascontiguousarray(Wqkv[:, 1 * C:2 * C][:, sl]),
            "wv": np.ascontiguousarray(Wqkv[:, 2 * C:3 * C][:, sl]),
            "bq": np.ascontiguousarray(bqkv[0 * C:1 * C][sl]),
            "bk": np.ascontiguousarray(bqkv[1 * C:2 * C][sl]),
            "bv": np.ascontiguousarray(bqkv[2 * C:3 * C][sl]),
            "wp": np.ascontiguousarray(Wproj[g * GC:(g + 1) * GC, :]),
            "pad": padding_mask[b].astype(np.float32),
            "mask": tril,
        })

    trace = bool(os.environ.get("BASS_KERNEL_TRACE"))
    res = run_bass_kernel_spmd(
        nc, in_maps, core_ids=list(range(N_CORES)), trace=trace)
    _cached["last_result"] = res

    out = np.empty((B, T, C), dtype=np.float32)
    for b in range(B):
        out[b] = res.results[2 * b]["out"] + res.results[2 * b + 1]["out"] + bproj
    return out
